# revision 5
# baseline (speedup 1.0000x reference)
import numpy as np
from contextlib import ExitStack

import concourse.bass as bass
import concourse.bacc as bacc
import concourse.mybir as mybir
from concourse.tile import TileContext
from concourse.bass_utils import run_bass_kernel_spmd

B, T, K, D = 512, 2048, 8, 32
DT = 0.05
NCORES = 8
BL = B // NCORES          # 64 paths per core
TC = 64                   # timesteps per chunk
NCH = T // TC
SW = 8                    # steps per bulk-gen block

F32 = mybir.dt.float32
F16 = mybir.dt.float16

_cache = {}

# G[p, 0:64]  = (R0aug^T z_aug)[p, b]   (k-half 0..3; includes b_s bias)
# G[p, 64:128]= (R1aug^T z_aug)[p, b]   (k-half 4..7)
# P = G * wnB (f16)        wnB[(k,i), (h,b)] = DT*wbar[k_h, b]
# G += C0^T P[:,0:64] + C0^T P[:,64:128]   (C0 = tile_k'(A0half): G_A update)
#   += C1^T ... for G_B;   G += R'^T dfn via dfn-moving matmuls
# Z (persistent [32, 64] f32 PSUM) += SR^T P halves + dfn;  ysT <- Z per step


def _build():
    nc = bacc.Bacc()
    z0T = nc.declare_dram_parameter("z0T", [D, BL], F32, isOutput=False)
    wT = nc.declare_dram_parameter("wT", [K, T, BL], F16, isOutput=False)
    nzT = nc.declare_dram_parameter("nzT", [D, T, BL], F16, isOutput=False)
    R0m = nc.declare_dram_parameter("R0m", [D + 1, 128], F32, isOutput=False)
    R1m = nc.declare_dram_parameter("R1m", [D + 1, 128], F32, isOutput=False)
    C0m = nc.declare_dram_parameter("C0m", [128, 128], F32, isOutput=False)
    C1m = nc.declare_dram_parameter("C1m", [128, 128], F32, isOutput=False)
    SAm = nc.declare_dram_parameter("SAm", [K, 128], F32, isOutput=False)
    SBm = nc.declare_dram_parameter("SBm", [K, 128], F32, isOutput=False)
    SRm = nc.declare_dram_parameter("SRm", [128, D], F32, isOutput=False)
    I32m = nc.declare_dram_parameter("I32m", [D, D], F32, isOutput=False)
    Qtm = nc.declare_dram_parameter("Qtm", [K, D], F32, isOutput=False)
    ysT = nc.declare_dram_parameter("ysT", [D, T, BL], F32, isOutput=True)

    ctx = ExitStack()
    with TileContext(nc) as tc:
        with (
            tc.tile_pool(name="const", bufs=1) as constp,
            tc.tile_pool(name="io", bufs=2) as iop,
            tc.tile_pool(name="work", bufs=2) as workp,
            tc.tile_pool(name="step", bufs=3) as stepp,
            tc.tile_pool(name="accg", bufs=1, space="PSUM") as accgp,
            tc.tile_pool(name="accz", bufs=1, space="PSUM") as acczp,
            tc.tile_pool(name="psw", bufs=2, space="PSUM") as pswp,
            tc.tile_pool(name="psd", bufs=2, space="PSUM") as psdp,
        ):
            def f16_const(name, dram, p, f):
                st = constp.tile([p, f], F32, tag=name + "32")
                nc.sync.dma_start(st[:], dram[:])
                cv = constp.tile([p, f], F16, tag=name)
                nc.vector.tensor_copy(cv[:], st[:])
                return cv

            R0 = f16_const("R0", R0m, D + 1, 128)
            R1 = f16_const("R1", R1m, D + 1, 128)
            C0 = f16_const("C0", C0m, 128, 128)
            C1 = f16_const("C1", C1m, 128, 128)
            SA = f16_const("SA", SAm, K, 128)
            SB = f16_const("SB", SBm, K, 128)
            SR = f16_const("SR", SRm, 128, D)
            I32 = f16_const("I32", I32m, D, D)
            Qt = f16_const("Qt", Qtm, K, D)

            z0_sb = constp.tile([D, BL], F32, tag="z0T")
            nc.sync.dma_start(z0_sb[:], z0T[:])
            # z0 aug (f16, ones row)
            z0aug = constp.tile([D + 1, BL], F16, tag="z0aug")
            nc.vector.memset(z0aug[D : D + 1, :], 1.0)
            nc.vector.tensor_copy(z0aug[0:D, :], z0_sb[:])

            # persistent accumulators.  G spans TWO banks: GA at the start
            # of bank 0, GB at the start of bank 1 (512 f32 in), so the two
            # start=True inits hit different zero regions, yet one strided
            # DVE access pattern [128, 2, 64] covers both halves.
            G2 = accgp.tile([128, 1024], F32, tag="G2")
            GAv = G2[:, 0:BL]
            GBv = G2[:, 512 : 512 + BL]
            Gview = G2[:].rearrange("p (h x) -> p h x", h=16)[:, 0::8, 0:BL]
            Z = acczp.tile([D, BL], F32, tag="Z")

            # G init = Raug^T z0aug ; Z init = z0 (via I32^T z0aug matmul)
            nc.tensor.matmul(GAv, R0[:], z0aug[:], start=True, stop=True,
                             skip_group_check=True)
            nc.tensor.matmul(GBv, R1[:], z0aug[:], start=True,
                             stop=True, skip_group_check=True)
            nc.tensor.matmul(Z[:], I32[:], z0aug[0:D, :], start=True, stop=True,
                             skip_group_check=True)

            for c in range(NCH):
                t0 = c * TC
                wT_ch = iop.tile([K, TC, BL], F16, tag="wT")
                nc.sync.dma_start(wT_ch[:], wT[:, t0 : t0 + TC, :])
                nzT_ch = iop.tile([D, TC, BL], F16, tag="nzT")
                nc.sync.dma_start(nzT_ch[:], nzT[:, t0 : t0 + TC, :])

                # bulk: wnB [(k,i), t, (h,b)] f16 and dfnT16 [d, t, b] f16
                wnB = workp.tile([128, TC, 2 * BL], F16, tag="wnB")
                dfnT = workp.tile([D, TC, BL], F16, tag="dfnT")
                for w in range(TC // SW):
                    mv = wT_ch[:, w * SW : (w + 1) * SW, :].rearrange(
                        "k t b -> k (t b)"
                    )
                    pswA = pswp.tile([128, SW * BL], F32, tag="psw")
                    nc.tensor.matmul(pswA[:], SA[:], mv, start=True, stop=True)
                    nc.scalar.copy(
                        wnB[:, w * SW : (w + 1) * SW, 0:BL],
                        pswA[:].rearrange("p (t b) -> p t b", b=BL),
                    )
                    pswB = pswp.tile([128, SW * BL], F32, tag="psw")
                    nc.tensor.matmul(pswB[:], SB[:], mv, start=True, stop=True)
                    nc.scalar.copy(
                        wnB[:, w * SW : (w + 1) * SW, BL : 2 * BL],
                        pswB[:].rearrange("p (t b) -> p t b", b=BL),
                    )
                    psD = psdp.tile([D, SW * BL], F32, tag="psD")
                    nc.tensor.matmul(psD[:], Qt[:], mv, start=True, stop=True)
                    nc.vector.tensor_mul(
                        dfnT[:, w * SW : (w + 1) * SW, :].rearrange(
                            "p t b -> p (t b)"
                        ),
                        psD[:],
                        nzT_ch[:, w * SW : (w + 1) * SW, :].rearrange(
                            "p t b -> p (t b)"
                        ),
                    )

                ysT_st = iop.tile([D, TC, BL], F32, tag="ysT")

                for s in range(TC):
                    # P = G * wnB: one DVE op spans both G banks
                    P = stepp.tile([128, 2 * BL], F16, tag="P")
                    nc.vector.tensor_mul(
                        P[:].rearrange("p (h x) -> p h x", h=2), Gview,
                        wnB[:, s, :].rearrange("p (h x) -> p h x", h=2),
                    )
                    Pa = P[:, 0:BL]
                    Pb = P[:, BL : 2 * BL]

                    dmv = dfnT[:, s, :]
                    # G += C^T (Pa+Pb) + R'^T dfn
                    nc.tensor.matmul(GAv, C0[:], Pa,
                                     start=False, stop=False, skip_group_check=True)
                    nc.tensor.matmul(GAv, C0[:], Pb,
                                     start=False, stop=False, skip_group_check=True)
                    nc.tensor.matmul(GAv, R0[0:D, :], dmv,
                                     start=False, stop=True, skip_group_check=True)
                    nc.tensor.matmul(GBv, C1[:], Pa,
                                     start=False, stop=False, skip_group_check=True)
                    nc.tensor.matmul(GBv, C1[:], Pb,
                                     start=False, stop=False, skip_group_check=True)
                    nc.tensor.matmul(GBv, R1[0:D, :], dmv,
                                     start=False, stop=True, skip_group_check=True)
                    # Z += SR^T P-halves + dfn
                    nc.tensor.matmul(Z[:], SR[:], Pa,
                                     start=False, stop=False, skip_group_check=True)
                    nc.tensor.matmul(Z[:], SR[:], Pb,
                                     start=False, stop=False, skip_group_check=True)
                    nc.tensor.matmul(Z[:], I32[:], dmv,
                                     start=False, stop=True, skip_group_check=True)
                    # snapshot state into the output chunk (ACT reads PSUM)
                    nc.scalar.copy(ysT_st[:, s, :], Z[:])

                nc.sync.dma_start(ysT[:, t0 : t0 + TC, :], ysT_st[:])
    ctx.close()
    nc.finalize()
    return nc


def _make_in_maps(z0, s_probs, noise, A_s, b_s, Q_chol):
    R0m = np.empty((D + 1, 128), np.float32)
    R0m[:D, :] = A_s[0:4].transpose(2, 0, 1).reshape(D, 128)
    R0m[D, :] = b_s[0:4].reshape(128)
    R1m = np.empty((D + 1, 128), np.float32)
    R1m[:D, :] = A_s[4:8].transpose(2, 0, 1).reshape(D, 128)
    R1m[D, :] = b_s[4:8].reshape(128)
    # C0[(k',j), (k,i)] = A[k, i, j]  (G_A-update composite, same for any k')
    C0m = np.tile(R0m[:D, :], (4, 1)).astype(np.float32)
    C1m = np.tile(R1m[:D, :], (4, 1)).astype(np.float32)
    SAm = np.zeros((K, 4, D), np.float32)
    SBm = np.zeros((K, 4, D), np.float32)
    for k in range(4):
        SAm[k, k, :] = DT
        SBm[k + 4, k, :] = DT
    SAm = SAm.reshape(K, 128)
    SBm = SBm.reshape(K, 128)
    SRm = np.tile(np.eye(D, dtype=np.float32), (4, 1))
    I32m = np.eye(D, dtype=np.float32)
    Qtm = (Q_chol * np.float32(np.sqrt(DT))).astype(np.float32)

    wsum = s_probs.sum(axis=-1, keepdims=True)
    wbarT = np.ascontiguousarray(
        (s_probs / wsum).transpose(2, 0, 1)
    ).astype(np.float16)
    nzT = np.ascontiguousarray(noise.transpose(2, 0, 1)).astype(np.float16)

    in_maps = []
    for c in range(NCORES):
        b0 = c * BL
        in_maps.append(
            {
                "z0T": np.ascontiguousarray(z0[b0 : b0 + BL].T),
                "wT": np.ascontiguousarray(wbarT[:, :, b0 : b0 + BL]),
                "nzT": np.ascontiguousarray(nzT[:, :, b0 : b0 + BL]),
                "R0m": R0m,
                "R1m": R1m,
                "C0m": C0m,
                "C1m": C1m,
                "SAm": SAm,
                "SBm": SBm,
                "SRm": SRm,
                "I32m": I32m,
                "Qtm": Qtm,
            }
        )
    return in_maps


def kernel(z0, s_probs, noise, A_s, b_s, Q_chol):
    if "nc" not in _cache:
        _cache["nc"] = _build()
    nc = _cache["nc"]

    A_s = np.asarray(A_s, np.float32)
    b_s = np.asarray(b_s, np.float32)
    Q_chol = np.asarray(Q_chol, np.float32)
    z0 = np.asarray(z0, np.float32)
    s_probs = np.asarray(s_probs, np.float32)
    noise = np.asarray(noise, np.float32)

    in_maps = _make_in_maps(z0, s_probs, noise, A_s, b_s, Q_chol)

    res = run_bass_kernel_spmd(nc, in_maps, list(range(NCORES))).results
    out = np.empty((T, B, D), np.float32)
    for c in range(NCORES):
        out[:, c * BL : (c + 1) * BL, :] = res[c]["ysT"].transpose(1, 2, 0)
    return out
